# revision 10
# baseline (speedup 1.0000x reference)
"""MultiHeadSelfAttention Trainium2 kernel (8 NeuronCores, SPMD).

Problem: x[2,2048,1024], H=16 heads, hd=64.  out = softmax(QK^T/8)V + x.

Sharding (tensor-parallel over heads x data-parallel over batch):
  core c (0..7): batch b = c//4, head group g = c%4 -> heads [4g, 4g+4),
  i.e. output columns [256g, 256g+256) of batch b.  No collectives: each
  core writes its own [2048, 256] slice; host concatenates.

Per-core layout/dataflow (everything fp32 in HBM; matmuls run as
float32r = full-rate; attn weights and V cast to bf16 for the AV matmul):
  host passes x[b]^T as `xT` [1024, 2048] (layout prep, not compute)
  Q^T, K^T:  [256(dh), 2048] = Wslice^T-free matmuls, lhsT=W tile, rhs=xT
  V:         [2048, 256] token-major, lhsT=xT tile, rhs=Wv (ones col added
             per head -> AV also produces sum(exp) for free)
  per head h, per 512-query block:
    S^T[k, q] = K_h @ Q_h^T     (16 k-tiles of 128, contraction=64)
    expS = exp(S^T / 8)         (ScalarE, fused scale, no max-subtraction:
                                 scores are O(1) for this input distribution)
    outT[65, q] = [V_h | 1]^T-matmul accumulating over k-tiles
    PE-transpose outT -> [q, 65]; divide by col 64 (sumexp); residual + bv.
"""

import numpy as np

B, S, D, H = 2, 2048, 1024, 16
HD = 64
NCORES = 8
GH = 4            # heads per core
GD = GH * HD      # 256 output columns per core
P = 128
DT = D // P       # 8 D-tiles (contraction)
KT = S // P       # 16 k-tiles
QB = 512          # query block
NQB = S // QB     # 4
NQT = S // P      # 16 query tiles of 128

_CACHE = {}
TRACE = False
LAST_RESULTS = None


def _build_nc(debug=False):
    import concourse.bass as bass
    import concourse.mybir as mybir
    import concourse.tile as tile
    from concourse import bacc
    from concourse.masks import make_identity

    f32 = mybir.dt.float32
    f32r = mybir.dt.float32r
    bf16 = mybir.dt.bfloat16
    EXP = mybir.ActivationFunctionType.Exp

    nc = bacc.Bacc("TRN2")

    # wq|wk|wv|x^T packed into one DRAM tensor so the first consumer
    # matmuls depend on a single DMA completion (walrus limits the number
    # of sync waits a (self-loading fp32r) matmul can carry).
    xw_d = nc.dram_tensor("xw", [D, 3 * GD + S], f32r, kind="ExternalInput")
    bq_d = nc.dram_tensor("bq", [GD], f32, kind="ExternalInput")
    bk_d = nc.dram_tensor("bk", [GD], f32, kind="ExternalInput")
    bv_d = nc.dram_tensor("bv", [GD], f32, kind="ExternalInput")
    xres_d = nc.dram_tensor("xres", [S, GD], f32, kind="ExternalInput")
    out_d = nc.dram_tensor("out", [S, GD], f32, kind="ExternalOutput")
    if debug:
        dbg_q = nc.dram_tensor("dbg_q", [P, 2, S], f32r, kind="ExternalOutput")
        dbg_k = nc.dram_tensor("dbg_k", [P, 2, S], f32r, kind="ExternalOutput")
        dbg_v = nc.dram_tensor("dbg_v", [P, KT, GH * (HD + 1)], bf16, kind="ExternalOutput")
        dbg_e = nc.dram_tensor("dbg_e", [P, KT, QB], bf16, kind="ExternalOutput")
        dbg_o = nc.dram_tensor("dbg_o", [HD + 1, QB], f32, kind="ExternalOutput")

    with tile.TileContext(nc) as tc:
        with (
            tc.tile_pool(name="persist", bufs=1) as persist,
            tc.tile_pool(name="exps_pool", bufs=2) as exps_pool,
            tc.tile_pool(name="work", bufs=3) as work,
            tc.tile_pool(name="psum", bufs=2, space="PSUM") as psum,
        ):
            # ---- constants / weights ----
            identity = persist.tile([P, P], f32, tag="identity")
            make_identity(nc, identity)

            bq_sb = persist.tile([P, 2], f32, tag="bq_sb")
            nc.sync.dma_start(bq_sb, bq_d.rearrange("(m p) -> p m", p=P))
            bk_sb = persist.tile([P, 2], f32, tag="bk_sb")
            nc.sync.dma_start(bk_sb, bk_d.rearrange("(m p) -> p m", p=P))

            bv_bc = persist.tile([P, GD], f32, tag="bv_bc")
            bv_ap = bass.AP(
                tensor=bv_d[:].tensor, offset=bv_d[:].offset,
                ap=[[0, P]] + list(bv_d[:].ap),
            )
            nc.gpsimd.dma_start(out=bv_bc, in_=bv_ap)

            # ---- weights + x^T, first chunk carries the weights ----
            xw_sb = persist.tile([P, DT, 3 * GD + S], f32r, tag="xw_sb")
            xw_r = xw_d.rearrange("(dt p) s -> p dt s", p=P)
            W0 = 3 * GD
            bounds = [0, W0 + QB, W0 + 2 * QB, W0 + 3 * QB, W0 + S]
            for c in range(4):
                nc.sync.dma_start(
                    xw_sb[:, :, bounds[c]:bounds[c + 1]],
                    xw_r[:, :, bounds[c]:bounds[c + 1]],
                )
            wq_sb = xw_sb[:, :, 0:GD]
            wk_sb = xw_sb[:, :, GD:2 * GD]
            wv_sb = xw_sb[:, :, 2 * GD:3 * GD]
            xT_sb = xw_sb[:, :, 3 * GD:]

            # Pre-observe the small constant DMAs on DVE with dummy reads, so
            # downstream DVE consumers (TT/TS instruction words have only one
            # sync-wait slot) never carry a DMA wait alongside a PE wait.
            sink = persist.tile([P, 4], f32, tag="sink")
            nc.vector.tensor_copy(sink[:, 0:1], bv_bc[:, 0:1])
            nc.vector.tensor_copy(sink[:, 1:2], bq_sb[:, 0:1])
            nc.vector.tensor_copy(sink[:, 2:3], bk_sb[:, 0:1])

            # ---- persistent activations ----
            qT_sb = persist.tile([P, 2, S], f32r, tag="qT_sb")
            kT_sb = persist.tile([P, 2, S], f32r, tag="kT_sb")
            v_sb = persist.tile([P, KT, GH * (HD + 1)], bf16, tag="v_sb")
            out_sb = persist.tile([P, NQT, GD], f32, tag="out_sb")

            # ones columns for the sum(exp) trick
            ones_view = v_sb.rearrange("p t (h c) -> p t h c", c=HD + 1)[:, :, :, HD:]
            nc.vector.memset(ones_view, 1.0)

            def emit_qk(m):
                """Q^T / K^T projection for M-tile m (heads 2m, 2m+1)."""
                for w_sb, b_sb, dst in ((wq_sb, bq_sb, qT_sb), (wk_sb, bk_sb, kT_sb)):
                    for q2 in range(2):          # 1024-token chunks
                        ps = psum.tile([P, 1024], f32, tag="ps_big", name="ps_proj")
                        for half in range(2):
                            tok = (q2 * 2 + half) * QB
                            for dt in range(DT):
                                nc.tensor.matmul(
                                    ps[:, half * QB:(half + 1) * QB],
                                    lhsT=w_sb[:, dt, m * P:(m + 1) * P],
                                    rhs=xT_sb[:, dt, tok:tok + QB],
                                    start=(dt == 0), stop=(dt == DT - 1),
                                )
                        nc.vector.tensor_scalar_add(
                            dst[:, m, q2 * 1024:(q2 + 1) * 1024], ps, b_sb[:, m:m + 1],
                        )

            def emit_v():
                for tt in range(KT):
                    ps = psum.tile([P, GD], f32, tag="ps_small", name="ps_v")
                    for dt in range(DT):
                        nc.tensor.matmul(
                            ps,
                            lhsT=xT_sb[:, dt, tt * P:(tt + 1) * P],
                            rhs=wv_sb[:, dt, :],
                            start=(dt == 0), stop=(dt == DT - 1),
                        )
                    nc.vector.tensor_tensor(
                        v_sb[:, tt, :].rearrange("p (h c) -> p h c", c=HD + 1)[:, :, :HD],
                        ps.rearrange("p (h c) -> p h c", c=HD),
                        bv_bc.rearrange("p (h c) -> p h c", c=HD),
                        mybir.AluOpType.add,
                    )

            def emit_head(hh):
                """Attention for core-local head hh (0..3)."""
                m, base = hh // 2, (hh % 2) * HD
                vcol = hh * (HD + 1)
                for qb in range(NQB):
                    q0 = qb * QB
                    exps = exps_pool.tile([P, KT, QB], bf16, tag="exps", name="exps")
                    for kt2 in range(KT // 2):
                        pss = psum.tile([P, 1024], f32, tag="ps_big", name="ps_s")
                        for half in range(2):
                            kt = 2 * kt2 + half
                            nc.tensor.matmul(
                                pss[:, half * QB:(half + 1) * QB],
                                lhsT=kT_sb[base:base + HD, m, kt * P:(kt + 1) * P],
                                rhs=qT_sb[base:base + HD, m, q0:q0 + QB],
                                start=True, stop=True,
                            )
                        nc.scalar.activation(
                            out=exps[:, 2 * kt2:2 * kt2 + 2, :],
                            in_=pss, func=EXP, scale=0.125,
                        )
                    pso = psum.tile([HD + 1, QB], f32, tag="ps_small", name="ps_o")
                    for kt in range(KT):
                        nc.tensor.matmul(
                            pso,
                            lhsT=v_sb[:, kt, vcol:vcol + HD + 1],
                            rhs=exps[:, kt, :],
                            start=(kt == 0), stop=(kt == KT - 1),
                        )
                    oT = work.tile([HD + 1, QB], f32, tag="oT", name="oT")
                    nc.vector.tensor_copy(oT, pso)
                    if debug and hh == 0 and qb == 0:
                        nc.sync.dma_start(dbg_e[:, :, :], exps)
                        nc.sync.dma_start(dbg_o[:, :], oT)
                    for q4 in range(QB // P):
                        qt = qb * (QB // P) + q4
                        pst = psum.tile([P, HD + 1], f32, tag="ps_small", name="ps_t")
                        nc.tensor.transpose(
                            pst, oT[:, q4 * P:(q4 + 1) * P],
                            identity[:HD + 1, :HD + 1],
                        )
                        r = work.tile([P, 1], f32, tag="recip", name="recip")
                        nc.vector.reciprocal(r, pst[:, HD:HD + 1])
                        nc.vector.tensor_scalar_mul(
                            out_sb[:, qt, hh * HD:(hh + 1) * HD], pst[:, :HD], r,
                        )

            if debug:
                pass  # debug stores emitted inline below

            # order: QK for heads 0/1 + V first, attention 0/1 overlaps QK 2/3
            emit_qk(0)
            emit_v()
            emit_head(0)
            emit_qk(1)
            emit_head(1)
            emit_head(2)
            emit_head(3)

            if debug:
                nc.sync.dma_start(dbg_q[:, :, :], qT_sb)
                nc.sync.dma_start(dbg_k[:, :, :], kT_sb)
                nc.sync.dma_start(dbg_v[:, :, :], v_sb)

            # ---- residual + bias + store ----
            for qt in range(NQT):
                xr = work.tile([P, GD], f32, tag="xr", name="xr")
                nc.sync.dma_start(xr, xres_d[qt * P:(qt + 1) * P, :])
                nc.vector.tensor_add(out_sb[:, qt, :], out_sb[:, qt, :], xr)
                nc.sync.dma_start(out_d[qt * P:(qt + 1) * P, :], out_sb[:, qt, :])

    nc.finalize()
    return nc


def _get_nc(debug=False):
    key = "nc_dbg" if debug else "nc"
    if key not in _CACHE:
        _CACHE[key] = _build_nc(debug=debug)
    return _CACHE[key]


def _round_fp32r(a):
    """Round fp32 to the fp32r grid (11-bit mantissa; low 12 bits dropped,
    round-half-up) so DMA'd data matches what the PE consumes as fp32r."""
    u = np.ascontiguousarray(a, dtype=np.float32).view(np.uint32)
    r = ((u.astype(np.uint64) + 0x800) & 0xFFFFF000).astype(np.uint32)
    return r.view(np.float32)


def kernel(x, Wq, bq, Wk, bk, Wv, bv):
    global LAST_RESULTS
    from concourse.bass_utils import run_bass_kernel_spmd

    x = np.asarray(x, dtype=np.float32)
    Wq, Wk, Wv = (np.asarray(a, dtype=np.float32) for a in (Wq, Wk, Wv))
    bq, bk, bv = (np.asarray(a, dtype=np.float32) for a in (bq, bk, bv))

    xTs = [x[b].T for b in range(B)]
    in_maps = []
    for c in range(NCORES):
        b, g = c // 4, c % 4
        cols = slice(GD * g, GD * (g + 1))
        xw = _round_fp32r(np.concatenate(
            [Wq[:, cols], Wk[:, cols], Wv[:, cols], xTs[b]], axis=1))
        in_maps.append({
            "xw": xw,
            "bq": np.ascontiguousarray(bq[cols]),
            "bk": np.ascontiguousarray(bk[cols]),
            "bv": np.ascontiguousarray(bv[cols]),
            "xres": np.ascontiguousarray(x[b][:, cols]),
        })

    nc = _get_nc()
    res = run_bass_kernel_spmd(
        nc, in_maps, core_ids=list(range(NCORES)), trace=TRACE,
    )
    LAST_RESULTS = res

    full = np.empty((B, S, D), dtype=np.float32)
    for c in range(NCORES):
        b, g = c // 4, c % 4
        full[b, :, GD * g:GD * (g + 1)] = res.results[c]["out"]
    return full


# revision 11
# speedup vs baseline: 1.0944x; 1.0944x over previous
"""MultiHeadSelfAttention Trainium2 kernel (8 NeuronCores, SPMD).

Problem: x[2,2048,1024], H=16 heads, hd=64.  out = softmax(QK^T/8)V + x.

Sharding (tensor-parallel over heads x data-parallel over batch):
  core c (0..7): batch b = c//4, head group g = c%4 -> heads [4g, 4g+4),
  i.e. output columns [256g, 256g+256) of batch b.  No collectives: each
  core writes its own [2048, 256] slice; host concatenates.

Per-core layout/dataflow (everything fp32 in HBM; matmuls run as
float32r = full-rate; attn weights and V cast to bf16 for the AV matmul):
  host passes x[b]^T as `xT` [1024, 2048] (layout prep, not compute)
  Q^T, K^T:  [256(dh), 2048] = Wslice^T-free matmuls, lhsT=W tile, rhs=xT
  V:         [2048, 256] token-major, lhsT=xT tile, rhs=Wv (ones col added
             per head -> AV also produces sum(exp) for free)
  per head h, per 512-query block:
    S^T[k, q] = K_h @ Q_h^T     (16 k-tiles of 128, contraction=64)
    expS = exp(S^T / 8)         (ScalarE, fused scale, no max-subtraction:
                                 scores are O(1) for this input distribution)
    outT[65, q] = [V_h | 1]^T-matmul accumulating over k-tiles
    PE-transpose outT -> [q, 65]; divide by col 64 (sumexp); residual + bv.
"""

import ml_dtypes
import numpy as np

B, S, D, H = 2, 2048, 1024, 16
HD = 64
NCORES = 8
GH = 4            # heads per core
GD = GH * HD      # 256 output columns per core
P = 128
DT = D // P       # 8 D-tiles (contraction)
KT = S // P       # 16 k-tiles
QB = 512          # query block
NQB = S // QB     # 4
NQT = S // P      # 16 query tiles of 128

_CACHE = {}
TRACE = False
LAST_RESULTS = None


def _build_nc(debug=False):
    import concourse.bass as bass
    import concourse.mybir as mybir
    import concourse.tile as tile
    from concourse import bacc
    from concourse.masks import make_identity

    f32 = mybir.dt.float32
    f32r = mybir.dt.float32r
    bf16 = mybir.dt.bfloat16
    EXP = mybir.ActivationFunctionType.Exp

    nc = bacc.Bacc("TRN2")

    # wq|wk|wv|x^T packed into one DRAM tensor so the first consumer
    # matmuls depend on a single DMA completion (walrus limits the number
    # of sync waits a matmul can carry). bf16: fp32(r) matmuls run in
    # fp32_mode=HIGH which neither hits 1cyc/row nor warms the PE HAM
    # clock gate -- measured 3x slower than bf16 end-to-end.
    xw_d = nc.dram_tensor("xw", [D, 3 * GD + S], bf16, kind="ExternalInput")
    bq_d = nc.dram_tensor("bq", [GD], f32, kind="ExternalInput")
    bk_d = nc.dram_tensor("bk", [GD], f32, kind="ExternalInput")
    bv_d = nc.dram_tensor("bv", [GD], f32, kind="ExternalInput")
    xres_d = nc.dram_tensor("xres", [S, GD], f32, kind="ExternalInput")
    out_d = nc.dram_tensor("out", [S, GD], f32, kind="ExternalOutput")
    if debug:
        dbg_q = nc.dram_tensor("dbg_q", [P, 2, S], bf16, kind="ExternalOutput")
        dbg_k = nc.dram_tensor("dbg_k", [P, 2, S], bf16, kind="ExternalOutput")
        dbg_v = nc.dram_tensor("dbg_v", [P, KT, GH * (HD + 1)], bf16, kind="ExternalOutput")
        dbg_e = nc.dram_tensor("dbg_e", [P, KT, QB], bf16, kind="ExternalOutput")
        dbg_o = nc.dram_tensor("dbg_o", [HD + 1, QB], f32, kind="ExternalOutput")

    with tile.TileContext(nc) as tc:
        with (
            tc.tile_pool(name="persist", bufs=1) as persist,
            tc.tile_pool(name="exps_pool", bufs=2) as exps_pool,
            tc.tile_pool(name="work", bufs=3) as work,
            tc.tile_pool(name="psum", bufs=2, space="PSUM") as psum,
        ):
            # ---- constants / weights ----
            identity = persist.tile([P, P], f32, tag="identity")
            make_identity(nc, identity)

            bq_sb = persist.tile([P, 2], f32, tag="bq_sb")
            nc.sync.dma_start(bq_sb, bq_d.rearrange("(m p) -> p m", p=P))
            bk_sb = persist.tile([P, 2], f32, tag="bk_sb")
            nc.sync.dma_start(bk_sb, bk_d.rearrange("(m p) -> p m", p=P))

            bv_bc = persist.tile([P, GD], f32, tag="bv_bc")
            bv_ap = bass.AP(
                tensor=bv_d[:].tensor, offset=bv_d[:].offset,
                ap=[[0, P]] + list(bv_d[:].ap),
            )
            nc.gpsimd.dma_start(out=bv_bc, in_=bv_ap)

            # ---- weights + x^T, first chunk carries the weights ----
            xw_sb = persist.tile([P, DT, 3 * GD + S], bf16, tag="xw_sb")
            xw_r = xw_d.rearrange("(dt p) s -> p dt s", p=P)
            W0 = 3 * GD
            bounds = [0, W0 + QB, W0 + 2 * QB, W0 + 3 * QB, W0 + S]
            for c in range(4):
                nc.sync.dma_start(
                    xw_sb[:, :, bounds[c]:bounds[c + 1]],
                    xw_r[:, :, bounds[c]:bounds[c + 1]],
                )
            wq_sb = xw_sb[:, :, 0:GD]
            wk_sb = xw_sb[:, :, GD:2 * GD]
            wv_sb = xw_sb[:, :, 2 * GD:3 * GD]
            xT_sb = xw_sb[:, :, 3 * GD:]

            # Pre-observe the small constant DMAs on DVE with dummy reads, so
            # downstream DVE consumers (TT/TS instruction words have only one
            # sync-wait slot) never carry a DMA wait alongside a PE wait.
            sink = persist.tile([P, 4], f32, tag="sink")
            nc.vector.tensor_copy(sink[:, 0:1], bv_bc[:, 0:1])
            nc.vector.tensor_copy(sink[:, 1:2], bq_sb[:, 0:1])
            nc.vector.tensor_copy(sink[:, 2:3], bk_sb[:, 0:1])

            # ---- persistent activations ----
            qT_sb = persist.tile([P, 2, S], bf16, tag="qT_sb")
            kT_sb = persist.tile([P, 2, S], bf16, tag="kT_sb")
            v_sb = persist.tile([P, KT, GH * (HD + 1)], bf16, tag="v_sb")
            out_sb = persist.tile([P, NQT, GD], f32, tag="out_sb")

            # ones columns for the sum(exp) trick
            ones_view = v_sb.rearrange("p t (h c) -> p t h c", c=HD + 1)[:, :, :, HD:]
            nc.vector.memset(ones_view, 1.0)

            def emit_qk(m):
                """Q^T / K^T projection for M-tile m (heads 2m, 2m+1)."""
                for w_sb, b_sb, dst in ((wq_sb, bq_sb, qT_sb), (wk_sb, bk_sb, kT_sb)):
                    for q2 in range(2):          # 1024-token chunks
                        ps = psum.tile([P, 1024], f32, tag="ps_big", name="ps_proj")
                        for half in range(2):
                            tok = (q2 * 2 + half) * QB
                            for dt in range(DT):
                                nc.tensor.matmul(
                                    ps[:, half * QB:(half + 1) * QB],
                                    lhsT=w_sb[:, dt, m * P:(m + 1) * P],
                                    rhs=xT_sb[:, dt, tok:tok + QB],
                                    start=(dt == 0), stop=(dt == DT - 1),
                                )
                        nc.vector.tensor_scalar_add(
                            dst[:, m, q2 * 1024:(q2 + 1) * 1024], ps, b_sb[:, m:m + 1],
                        )

            def emit_v():
                for tt in range(KT):
                    ps = psum.tile([P, GD], f32, tag="ps_small", name="ps_v")
                    for dt in range(DT):
                        nc.tensor.matmul(
                            ps,
                            lhsT=xT_sb[:, dt, tt * P:(tt + 1) * P],
                            rhs=wv_sb[:, dt, :],
                            start=(dt == 0), stop=(dt == DT - 1),
                        )
                    nc.vector.tensor_tensor(
                        v_sb[:, tt, :].rearrange("p (h c) -> p h c", c=HD + 1)[:, :, :HD],
                        ps.rearrange("p (h c) -> p h c", c=HD),
                        bv_bc.rearrange("p (h c) -> p h c", c=HD),
                        mybir.AluOpType.add,
                    )

            def emit_head(hh):
                """Attention for core-local head hh (0..3)."""
                m, base = hh // 2, (hh % 2) * HD
                vcol = hh * (HD + 1)
                for qb in range(NQB):
                    q0 = qb * QB
                    exps = exps_pool.tile([P, KT, QB], bf16, tag="exps", name="exps")
                    for kt2 in range(KT // 2):
                        pss = psum.tile([P, 1024], f32, tag="ps_big", name="ps_s")
                        for half in range(2):
                            kt = 2 * kt2 + half
                            nc.tensor.matmul(
                                pss[:, half * QB:(half + 1) * QB],
                                lhsT=kT_sb[base:base + HD, m, kt * P:(kt + 1) * P],
                                rhs=qT_sb[base:base + HD, m, q0:q0 + QB],
                                start=True, stop=True,
                            )
                        nc.scalar.activation(
                            out=exps[:, 2 * kt2:2 * kt2 + 2, :],
                            in_=pss, func=EXP, scale=0.125,
                        )
                    pso = psum.tile([HD + 1, QB], f32, tag="ps_small", name="ps_o")
                    for kt in range(KT):
                        nc.tensor.matmul(
                            pso,
                            lhsT=v_sb[:, kt, vcol:vcol + HD + 1],
                            rhs=exps[:, kt, :],
                            start=(kt == 0), stop=(kt == KT - 1),
                        )
                    oT = work.tile([HD + 1, QB], f32, tag="oT", name="oT")
                    nc.vector.tensor_copy(oT, pso)
                    if debug and hh == 0 and qb == 0:
                        nc.sync.dma_start(dbg_e[:, :, :], exps)
                        nc.sync.dma_start(dbg_o[:, :], oT)
                    for q4 in range(QB // P):
                        qt = qb * (QB // P) + q4
                        pst = psum.tile([P, HD + 1], f32, tag="ps_small", name="ps_t")
                        nc.tensor.transpose(
                            pst, oT[:, q4 * P:(q4 + 1) * P],
                            identity[:HD + 1, :HD + 1],
                        )
                        r = work.tile([P, 1], f32, tag="recip", name="recip")
                        nc.vector.reciprocal(r, pst[:, HD:HD + 1])
                        nc.vector.tensor_scalar_mul(
                            out_sb[:, qt, hh * HD:(hh + 1) * HD], pst[:, :HD], r,
                        )

            if debug:
                pass  # debug stores emitted inline below

            # order: QK for heads 0/1 + V first, attention 0/1 overlaps QK 2/3
            emit_qk(0)
            emit_v()
            emit_head(0)
            emit_qk(1)
            emit_head(1)
            emit_head(2)
            emit_head(3)

            if debug:
                nc.sync.dma_start(dbg_q[:, :, :], qT_sb)
                nc.sync.dma_start(dbg_k[:, :, :], kT_sb)
                nc.sync.dma_start(dbg_v[:, :, :], v_sb)

            # ---- residual + bias + store ----
            for qt in range(NQT):
                xr = work.tile([P, GD], f32, tag="xr", name="xr")
                nc.sync.dma_start(xr, xres_d[qt * P:(qt + 1) * P, :])
                nc.vector.tensor_add(out_sb[:, qt, :], out_sb[:, qt, :], xr)
                nc.sync.dma_start(out_d[qt * P:(qt + 1) * P, :], out_sb[:, qt, :])

    nc.finalize()
    return nc


def _get_nc(debug=False):
    key = "nc_dbg" if debug else "nc"
    if key not in _CACHE:
        _CACHE[key] = _build_nc(debug=debug)
    return _CACHE[key]


def _round_fp32r(a):
    """Round fp32 to the fp32r grid (11-bit mantissa; low 12 bits dropped,
    round-half-up) so DMA'd data matches what the PE consumes as fp32r."""
    u = np.ascontiguousarray(a, dtype=np.float32).view(np.uint32)
    r = ((u.astype(np.uint64) + 0x800) & 0xFFFFF000).astype(np.uint32)
    return r.view(np.float32)


def kernel(x, Wq, bq, Wk, bk, Wv, bv):
    global LAST_RESULTS
    from concourse.bass_utils import run_bass_kernel_spmd

    x = np.asarray(x, dtype=np.float32)
    Wq, Wk, Wv = (np.asarray(a, dtype=np.float32) for a in (Wq, Wk, Wv))
    bq, bk, bv = (np.asarray(a, dtype=np.float32) for a in (bq, bk, bv))

    xTs = [x[b].T for b in range(B)]
    in_maps = []
    for c in range(NCORES):
        b, g = c // 4, c % 4
        cols = slice(GD * g, GD * (g + 1))
        xw = np.concatenate(
            [Wq[:, cols], Wk[:, cols], Wv[:, cols], xTs[b]],
            axis=1).astype(ml_dtypes.bfloat16)
        in_maps.append({
            "xw": xw,
            "bq": np.ascontiguousarray(bq[cols]),
            "bk": np.ascontiguousarray(bk[cols]),
            "bv": np.ascontiguousarray(bv[cols]),
            "xres": np.ascontiguousarray(x[b][:, cols]),
        })

    nc = _get_nc()
    res = run_bass_kernel_spmd(
        nc, in_maps, core_ids=list(range(NCORES)), trace=TRACE,
    )
    LAST_RESULTS = res

    full = np.empty((B, S, D), dtype=np.float32)
    for c in range(NCORES):
        b, g = c // 4, c % 4
        full[b, :, GD * g:GD * (g + 1)] = res.results[c]["out"]
    return full


# revision 15
# speedup vs baseline: 1.6396x; 1.4982x over previous
"""MultiHeadSelfAttention Trainium2 kernel (8 NeuronCores, SPMD).

Problem: x[2,2048,1024], H=16 heads, hd=64.  out = softmax(QK^T/8)V + x.

Sharding (tensor-parallel over heads x data-parallel over batch):
  core c (0..7): batch b = c//4, head group g = c%4 -> heads [4g, 4g+4),
  i.e. output columns [256g, 256g+256) of batch b.  No collectives: each
  core writes its own [2048, 256] slice; host concatenates.

Per-core layout/dataflow (everything fp32 in HBM; matmuls run as
float32r = full-rate; attn weights and V cast to bf16 for the AV matmul):
  host passes x[b]^T as `xT` [1024, 2048] (layout prep, not compute)
  Q^T, K^T:  [256(dh), 2048] = Wslice^T-free matmuls, lhsT=W tile, rhs=xT
  V:         [2048, 256] token-major, lhsT=xT tile, rhs=Wv (ones col added
             per head -> AV also produces sum(exp) for free)
  per head h, per 512-query block:
    S^T[k, q] = K_h @ Q_h^T     (16 k-tiles of 128, contraction=64)
    expS = exp(S^T / 8)         (ScalarE, fused scale, no max-subtraction:
                                 scores are O(1) for this input distribution)
    outT[65, q] = [V_h | 1]^T-matmul accumulating over k-tiles
    PE-transpose outT -> [q, 65]; divide by col 64 (sumexp); residual + bv.
"""

import ml_dtypes
import numpy as np

B, S, D, H = 2, 2048, 1024, 16
HD = 64
NCORES = 8
GH = 4            # heads per core
GD = GH * HD      # 256 output columns per core
P = 128
DT = D // P       # 8 D-tiles (contraction)
KT = S // P       # 16 k-tiles
QB = 512          # query block
NQB = S // QB     # 4
NQT = S // P      # 16 query tiles of 128

_CACHE = {}
TRACE = False
LAST_RESULTS = None


def _build_nc(debug=False):
    import concourse.bass as bass
    import concourse.mybir as mybir
    import concourse.tile as tile
    from concourse import bacc
    from concourse.masks import make_identity

    f32 = mybir.dt.float32
    f32r = mybir.dt.float32r
    bf16 = mybir.dt.bfloat16
    EXP = mybir.ActivationFunctionType.Exp

    nc = bacc.Bacc("TRN2")

    # wq|wk|wv|x^T packed into one DRAM tensor so the first consumer
    # matmuls depend on a single DMA completion (walrus limits the number
    # of sync waits a matmul can carry). bf16: fp32(r) matmuls run in
    # fp32_mode=HIGH which neither hits 1cyc/row nor warms the PE HAM
    # clock gate -- measured 3x slower than bf16 end-to-end.
    xw_d = nc.dram_tensor("xw", [D, 3 * GD + S], bf16, kind="ExternalInput")
    bq_d = nc.dram_tensor("bq", [GD], f32, kind="ExternalInput")
    bk_d = nc.dram_tensor("bk", [GD], f32, kind="ExternalInput")
    bv_d = nc.dram_tensor("bv", [GD], f32, kind="ExternalInput")
    xres_d = nc.dram_tensor("xres", [S, GD], f32, kind="ExternalInput")
    out_d = nc.dram_tensor("out", [S, GD], f32, kind="ExternalOutput")
    if debug:
        dbg_q = nc.dram_tensor("dbg_q", [P, 2, S], bf16, kind="ExternalOutput")
        dbg_k = nc.dram_tensor("dbg_k", [P, GH, S], bf16, kind="ExternalOutput")
        dbg_v = nc.dram_tensor("dbg_v", [P, KT, GH * (HD + 1)], bf16, kind="ExternalOutput")
        dbg_e = nc.dram_tensor("dbg_e", [P, KT, QB], bf16, kind="ExternalOutput")
        dbg_o = nc.dram_tensor("dbg_o", [HD + 1, QB], f32, kind="ExternalOutput")

    with tile.TileContext(nc) as tc:
        with (
            tc.tile_pool(name="persist", bufs=1) as persist,
            tc.tile_pool(name="exps_pool", bufs=2) as exps_pool,
            tc.tile_pool(name="work", bufs=3) as work,
            tc.tile_pool(name="psum", bufs=2, space="PSUM") as psum,
        ):
            # ---- constants / weights ----
            identity = persist.tile([P, P], f32, tag="identity")
            make_identity(nc, identity)

            bq_sb = persist.tile([P, 2], f32, tag="bq_sb")
            nc.sync.dma_start(bq_sb, bq_d.rearrange("(m p) -> p m", p=P))
            bk_sb = persist.tile([P, 2], f32, tag="bk_sb")
            nc.sync.dma_start(bk_sb, bk_d.rearrange("(m p) -> p m", p=P))

            bv_bc = persist.tile([P, GD], f32, tag="bv_bc")
            bv_ap = bass.AP(
                tensor=bv_d[:].tensor, offset=bv_d[:].offset,
                ap=[[0, P]] + list(bv_d[:].ap),
            )
            nc.gpsimd.dma_start(out=bv_bc, in_=bv_ap)

            # ---- weights + x^T, first chunk carries the weights ----
            xw_sb = persist.tile([P, DT, 3 * GD + S], bf16, tag="xw_sb")
            xw_r = xw_d.rearrange("(dt p) s -> p dt s", p=P)
            W0 = 3 * GD
            bounds = [0, W0 + QB, W0 + 2 * QB, W0 + 3 * QB, W0 + S]
            for c in range(4):
                nc.sync.dma_start(
                    xw_sb[:, :, bounds[c]:bounds[c + 1]],
                    xw_r[:, :, bounds[c]:bounds[c + 1]],
                )
            wq_sb = xw_sb[:, :, 0:GD]
            wk_sb = xw_sb[:, :, GD:2 * GD]
            wv_sb = xw_sb[:, :, 2 * GD:3 * GD]
            xT_sb = xw_sb[:, :, 3 * GD:]

            # Pre-observe the small constant DMAs on DVE with dummy reads, so
            # downstream DVE consumers (TT/TS instruction words have only one
            # sync-wait slot) never carry a DMA wait alongside a PE wait.
            sink = persist.tile([P, 4], f32, tag="sink")
            nc.vector.tensor_copy(sink[:, 0:1], bv_bc[:, 0:1])
            nc.vector.tensor_copy(sink[:, 1:2], bq_sb[:, 0:1])
            nc.vector.tensor_copy(sink[:, 2:3], bk_sb[:, 0:1])

            # ---- persistent activations ----
            # qT: heads 2m/2m+1 packed on partition halves of M-tile m.
            # kT: one slot per head, other 64 partitions zeroed, so S^T
            # matmuls contract over the full 128 rows (half-width matmuls
            # don't register as activity for the PE HAM clock gate and the
            # whole attention phase runs at 1.2 GHz otherwise).  Q^T needs
            # no padding: its junk rows hit K's zeros.
            qT_sb = persist.tile([P, 2, S], bf16, tag="qT_sb")
            kT_sb = persist.tile([P, GH, S], bf16, tag="kT_sb")
            v_sb = persist.tile([P, KT, GH * (HD + 1)], bf16, tag="v_sb")
            out_sb = persist.tile([P, NQT, GD], f32, tag="out_sb")
            kT_q = kT_sb.rearrange("p (m two) s -> p m two s", two=2)
            nc.vector.memset(kT_q[HD:, :, 0, :], 0.0)   # even heads: rows 64+
            nc.vector.memset(kT_q[:HD, :, 1, :], 0.0)   # odd heads: rows 0-63

            # ones columns for the sum(exp) trick
            ones_view = v_sb.rearrange("p t (h c) -> p t h c", c=HD + 1)[:, :, :, HD:]
            nc.vector.memset(ones_view, 1.0)

            def emit_qk(m):
                """Q^T / K^T projection for M-tile m (heads 2m, 2m+1)."""
                for w_sb, b_sb, dst, split in (
                    (wq_sb, bq_sb, qT_sb, False), (wk_sb, bk_sb, kT_sb, True),
                ):
                    for q2 in range(2):          # 1024-token chunks
                        ps = psum.tile([P, 1024], f32, tag="ps_big", name="ps_proj")
                        for half in range(2):
                            tok = (q2 * 2 + half) * QB
                            for dt in range(DT):
                                nc.tensor.matmul(
                                    ps[:, half * QB:(half + 1) * QB],
                                    lhsT=w_sb[:, dt, m * P:(m + 1) * P],
                                    rhs=xT_sb[:, dt, tok:tok + QB],
                                    start=(dt == 0), stop=(dt == DT - 1),
                                )
                        sl = slice(q2 * 1024, (q2 + 1) * 1024)
                        if split:
                            # per-head slots; each head's data stays on its
                            # own partition half, the other half is zero
                            nc.vector.tensor_scalar_add(
                                dst[:HD, 2 * m, sl], ps[:HD], b_sb[:HD, m:m + 1],
                            )
                            nc.vector.tensor_scalar_add(
                                dst[HD:, 2 * m + 1, sl], ps[HD:], b_sb[HD:, m:m + 1],
                            )
                        else:
                            nc.vector.tensor_scalar_add(
                                dst[:, m, sl], ps, b_sb[:, m:m + 1],
                            )

            def emit_v():
                for tt in range(KT):
                    ps = psum.tile([P, GD], f32, tag="ps_small", name="ps_v")
                    for dt in range(DT):
                        nc.tensor.matmul(
                            ps,
                            lhsT=xT_sb[:, dt, tt * P:(tt + 1) * P],
                            rhs=wv_sb[:, dt, :],
                            start=(dt == 0), stop=(dt == DT - 1),
                        )
                    nc.vector.tensor_tensor(
                        v_sb[:, tt, :].rearrange("p (h c) -> p h c", c=HD + 1)[:, :, :HD],
                        ps.rearrange("p (h c) -> p h c", c=HD),
                        bv_bc.rearrange("p (h c) -> p h c", c=HD),
                        mybir.AluOpType.add,
                    )

            def emit_head(hh):
                """Attention for core-local head hh (0..3)."""
                m, base = hh // 2, (hh % 2) * HD
                vcol = hh * (HD + 1)
                for qb in range(NQB):
                    q0 = qb * QB
                    exps = exps_pool.tile([P, KT, QB], bf16, tag="exps", name="exps")
                    for kt2 in range(KT // 2):
                        pss = psum.tile([P, 1024], f32, tag="ps_big", name="ps_s")
                        for half in range(2):
                            kt = 2 * kt2 + half
                            nc.tensor.matmul(
                                pss[:, half * QB:(half + 1) * QB],
                                lhsT=kT_sb[:, hh, kt * P:(kt + 1) * P],
                                rhs=qT_sb[:, m, q0:q0 + QB],
                                start=True, stop=True,
                            )
                        nc.scalar.activation(
                            out=exps[:, 2 * kt2:2 * kt2 + 2, :],
                            in_=pss, func=EXP, scale=0.125,
                        )
                    pso = psum.tile([HD + 1, QB], f32, tag="ps_small", name="ps_o")
                    for kt in range(KT):
                        nc.tensor.matmul(
                            pso,
                            lhsT=v_sb[:, kt, vcol:vcol + HD + 1],
                            rhs=exps[:, kt, :],
                            start=(kt == 0), stop=(kt == KT - 1),
                        )
                    oT = work.tile([HD + 1, QB], f32, tag="oT", name="oT")
                    nc.vector.tensor_copy(oT, pso)
                    if debug and hh == 0 and qb == 0:
                        nc.sync.dma_start(dbg_e[:, :, :], exps)
                        nc.sync.dma_start(dbg_o[:, :], oT)
                    for q4 in range(QB // P):
                        qt = qb * (QB // P) + q4
                        pst = psum.tile([P, HD + 1], f32, tag="ps_small", name="ps_t")
                        nc.tensor.transpose(
                            pst, oT[:, q4 * P:(q4 + 1) * P],
                            identity[:HD + 1, :HD + 1],
                        )
                        r = work.tile([P, 1], f32, tag="recip", name="recip")
                        nc.vector.reciprocal(r, pst[:, HD:HD + 1])
                        nc.vector.tensor_scalar_mul(
                            out_sb[:, qt, hh * HD:(hh + 1) * HD], pst[:, :HD], r,
                        )

            if debug:
                pass  # debug stores emitted inline below

            # order: QK for heads 0/1 + V first, attention 0/1 overlaps QK 2/3
            emit_qk(0)
            emit_v()
            emit_head(0)
            emit_qk(1)
            emit_head(1)
            emit_head(2)
            emit_head(3)

            if debug:
                nc.sync.dma_start(dbg_q[:, :, :], qT_sb)
                nc.sync.dma_start(dbg_k[:, :, :], kT_sb)
                nc.sync.dma_start(dbg_v[:, :, :], v_sb)

            # ---- residual + bias + store ----
            for qt in range(NQT):
                xr = work.tile([P, GD], f32, tag="xr", name="xr")
                nc.sync.dma_start(xr, xres_d[qt * P:(qt + 1) * P, :])
                nc.vector.tensor_add(out_sb[:, qt, :], out_sb[:, qt, :], xr)
                nc.sync.dma_start(out_d[qt * P:(qt + 1) * P, :], out_sb[:, qt, :])

    nc.finalize()
    return nc


def _get_nc(debug=False):
    key = "nc_dbg" if debug else "nc"
    if key not in _CACHE:
        _CACHE[key] = _build_nc(debug=debug)
    return _CACHE[key]


def _round_fp32r(a):
    """Round fp32 to the fp32r grid (11-bit mantissa; low 12 bits dropped,
    round-half-up) so DMA'd data matches what the PE consumes as fp32r."""
    u = np.ascontiguousarray(a, dtype=np.float32).view(np.uint32)
    r = ((u.astype(np.uint64) + 0x800) & 0xFFFFF000).astype(np.uint32)
    return r.view(np.float32)


def kernel(x, Wq, bq, Wk, bk, Wv, bv):
    global LAST_RESULTS
    from concourse.bass_utils import run_bass_kernel_spmd

    x = np.asarray(x, dtype=np.float32)
    Wq, Wk, Wv = (np.asarray(a, dtype=np.float32) for a in (Wq, Wk, Wv))
    bq, bk, bv = (np.asarray(a, dtype=np.float32) for a in (bq, bk, bv))

    xTs = [x[b].T for b in range(B)]
    in_maps = []
    for c in range(NCORES):
        b, g = c // 4, c % 4
        cols = slice(GD * g, GD * (g + 1))
        xw = np.concatenate(
            [Wq[:, cols], Wk[:, cols], Wv[:, cols], xTs[b]],
            axis=1).astype(ml_dtypes.bfloat16)
        in_maps.append({
            "xw": xw,
            "bq": np.ascontiguousarray(bq[cols]),
            "bk": np.ascontiguousarray(bk[cols]),
            "bv": np.ascontiguousarray(bv[cols]),
            "xres": np.ascontiguousarray(x[b][:, cols]),
        })

    nc = _get_nc()
    res = run_bass_kernel_spmd(
        nc, in_maps, core_ids=list(range(NCORES)), trace=TRACE,
    )
    LAST_RESULTS = res

    full = np.empty((B, S, D), dtype=np.float32)
    for c in range(NCORES):
        b, g = c // 4, c % 4
        full[b, :, GD * g:GD * (g + 1)] = res.results[c]["out"]
    return full


# revision 18
# speedup vs baseline: 1.7217x; 1.0500x over previous
"""MultiHeadSelfAttention Trainium2 kernel (8 NeuronCores, SPMD).

Problem: x[2,2048,1024], H=16 heads, hd=64.  out = softmax(QK^T/8)V + x.

Sharding (tensor-parallel over heads x data-parallel over batch):
  core c (0..7): batch b = c//4, head group g = c%4 -> heads [4g, 4g+4),
  i.e. output columns [256g, 256g+256) of batch b.  No collectives: each
  core writes its own [2048, 256] slice; host concatenates.

Per-core layout/dataflow (everything fp32 in HBM; matmuls run as
float32r = full-rate; attn weights and V cast to bf16 for the AV matmul):
  host passes x[b]^T as `xT` [1024, 2048] (layout prep, not compute)
  Q^T, K^T:  [256(dh), 2048] = Wslice^T-free matmuls, lhsT=W tile, rhs=xT
  V:         [2048, 256] token-major, lhsT=xT tile, rhs=Wv (ones col added
             per head -> AV also produces sum(exp) for free)
  per head h, per 512-query block:
    S^T[k, q] = K_h @ Q_h^T     (16 k-tiles of 128, contraction=64)
    expS = exp(S^T / 8)         (ScalarE, fused scale, no max-subtraction:
                                 scores are O(1) for this input distribution)
    outT[65, q] = [V_h | 1]^T-matmul accumulating over k-tiles
    PE-transpose outT -> [q, 65]; divide by col 64 (sumexp); residual + bv.
"""

import ml_dtypes
import numpy as np

B, S, D, H = 2, 2048, 1024, 16
HD = 64
NCORES = 8
GH = 4            # heads per core
GD = GH * HD      # 256 output columns per core
P = 128
DT = D // P       # 8 D-tiles (contraction)
KT = S // P       # 16 k-tiles
QB = 512          # query block
NQB = S // QB     # 4
NQT = S // P      # 16 query tiles of 128

_CACHE = {}
TRACE = False
LAST_RESULTS = None


def _build_nc(debug=False):
    import concourse.bass as bass
    import concourse.mybir as mybir
    import concourse.tile as tile
    from concourse import bacc
    from concourse.masks import make_identity

    f32 = mybir.dt.float32
    f32r = mybir.dt.float32r
    bf16 = mybir.dt.bfloat16
    EXP = mybir.ActivationFunctionType.Exp

    nc = bacc.Bacc("TRN2")

    # wq|wk|wv|x^T packed into one DRAM tensor so the first consumer
    # matmuls depend on a single DMA completion (walrus limits the number
    # of sync waits a matmul can carry). bf16: fp32(r) matmuls run in
    # fp32_mode=HIGH which neither hits 1cyc/row nor warms the PE HAM
    # clock gate -- measured 3x slower than bf16 end-to-end.
    xw_d = nc.dram_tensor("xw", [D, 3 * GD + S], bf16, kind="ExternalInput")
    bq_d = nc.dram_tensor("bq", [GD], f32, kind="ExternalInput")
    bk_d = nc.dram_tensor("bk", [GD], f32, kind="ExternalInput")
    bv_d = nc.dram_tensor("bv", [GD], f32, kind="ExternalInput")
    xres_d = nc.dram_tensor("xres", [S, GD], f32, kind="ExternalInput")
    out_d = nc.dram_tensor("out", [S, GD], f32, kind="ExternalOutput")
    if debug:
        dbg_q = nc.dram_tensor("dbg_q", [P, 2, S], bf16, kind="ExternalOutput")
        dbg_k = nc.dram_tensor("dbg_k", [P, GH, S], bf16, kind="ExternalOutput")
        dbg_v = nc.dram_tensor("dbg_v", [P, KT, GH * (HD + 1)], bf16, kind="ExternalOutput")
        dbg_e = nc.dram_tensor("dbg_e", [P, KT, QB], bf16, kind="ExternalOutput")
        dbg_o = nc.dram_tensor("dbg_o", [HD + 1, QB], f32, kind="ExternalOutput")

    with tile.TileContext(nc) as tc:
        with (
            tc.tile_pool(name="persist", bufs=1) as persist,
            tc.tile_pool(name="exps_pool", bufs=3) as exps_pool,
            tc.tile_pool(name="work", bufs=3) as work,
            tc.tile_pool(name="psum", bufs=2, space="PSUM") as psum,
        ):
            # ---- constants / weights ----
            identity = persist.tile([P, P], f32, tag="identity")
            make_identity(nc, identity)

            bq_sb = persist.tile([P, 2], f32, tag="bq_sb")
            nc.sync.dma_start(bq_sb, bq_d.rearrange("(m p) -> p m", p=P))
            bk_sb = persist.tile([P, 2], f32, tag="bk_sb")
            nc.sync.dma_start(bk_sb, bk_d.rearrange("(m p) -> p m", p=P))

            bv_bc = persist.tile([P, GD], f32, tag="bv_bc")
            bv_ap = bass.AP(
                tensor=bv_d[:].tensor, offset=bv_d[:].offset,
                ap=[[0, P]] + list(bv_d[:].ap),
            )
            nc.gpsimd.dma_start(out=bv_bc, in_=bv_ap)

            # ---- weights + x^T, first chunk carries the weights ----
            xw_sb = persist.tile([P, DT, 3 * GD + S], bf16, tag="xw_sb")
            xw_r = xw_d.rearrange("(dt p) s -> p dt s", p=P)
            W0 = 3 * GD
            bounds = [0, W0 + QB, W0 + 2 * QB, W0 + 3 * QB, W0 + S]
            for c in range(4):
                nc.sync.dma_start(
                    xw_sb[:, :, bounds[c]:bounds[c + 1]],
                    xw_r[:, :, bounds[c]:bounds[c + 1]],
                )
            wq_sb = xw_sb[:, :, 0:GD]
            wk_sb = xw_sb[:, :, GD:2 * GD]
            wv_sb = xw_sb[:, :, 2 * GD:3 * GD]
            xT_sb = xw_sb[:, :, 3 * GD:]

            # Pre-observe the small constant DMAs on DVE with dummy reads, so
            # downstream DVE consumers (TT/TS instruction words have only one
            # sync-wait slot) never carry a DMA wait alongside a PE wait.
            sink = persist.tile([P, 4], f32, tag="sink")
            nc.vector.tensor_copy(sink[:, 0:1], bv_bc[:, 0:1])
            nc.vector.tensor_copy(sink[:, 1:2], bq_sb[:, 0:1])
            nc.vector.tensor_copy(sink[:, 2:3], bk_sb[:, 0:1])

            # ---- persistent activations ----
            # qT: heads 2m/2m+1 packed on partition halves of M-tile m.
            # kT: one slot per head, other 64 partitions zeroed, so S^T
            # matmuls contract over the full 128 rows (half-width matmuls
            # don't register as activity for the PE HAM clock gate and the
            # whole attention phase runs at 1.2 GHz otherwise).  Q^T needs
            # no padding: its junk rows hit K's zeros.
            qT_sb = persist.tile([P, 2, S], bf16, tag="qT_sb")
            kT_sb = persist.tile([P, GH, S], bf16, tag="kT_sb")
            v_sb = persist.tile([P, KT, GH * (HD + 1)], bf16, tag="v_sb")
            out_sb = persist.tile([P, NQT, GD], f32, tag="out_sb")
            kT_q = kT_sb.rearrange("p (m two) s -> p m two s", two=2)
            nc.vector.memset(kT_q[HD:, :, 0, :], 0.0)   # even heads: rows 64+
            nc.vector.memset(kT_q[:HD, :, 1, :], 0.0)   # odd heads: rows 0-63

            # ones columns for the sum(exp) trick
            ones_view = v_sb.rearrange("p t (h c) -> p t h c", c=HD + 1)[:, :, :, HD:]
            nc.vector.memset(ones_view, 1.0)

            def emit_qk(m):
                """Q^T / K^T projection for M-tile m (heads 2m, 2m+1)."""
                for w_sb, b_sb, dst, split in (
                    (wq_sb, bq_sb, qT_sb, False), (wk_sb, bk_sb, kT_sb, True),
                ):
                    for q2 in range(2):          # 1024-token chunks
                        ps = psum.tile([P, 1024], f32, tag="ps_big", bufs=3, name="ps_proj")
                        for half in range(2):
                            tok = (q2 * 2 + half) * QB
                            for dt in range(DT):
                                nc.tensor.matmul(
                                    ps[:, half * QB:(half + 1) * QB],
                                    lhsT=w_sb[:, dt, m * P:(m + 1) * P],
                                    rhs=xT_sb[:, dt, tok:tok + QB],
                                    start=(dt == 0), stop=(dt == DT - 1),
                                )
                        sl = slice(q2 * 1024, (q2 + 1) * 1024)
                        if split:
                            # per-head slots; each head's data stays on its
                            # own partition half, the other half is zero
                            nc.vector.tensor_scalar_add(
                                dst[:HD, 2 * m, sl], ps[:HD], b_sb[:HD, m:m + 1],
                            )
                            nc.vector.tensor_scalar_add(
                                dst[HD:, 2 * m + 1, sl], ps[HD:], b_sb[HD:, m:m + 1],
                            )
                        else:
                            nc.vector.tensor_scalar_add(
                                dst[:, m, sl], ps, b_sb[:, m:m + 1],
                            )

            def emit_v():
                for tt in range(KT):
                    ps = psum.tile([P, GD], f32, tag="ps_small", name="ps_v")
                    for dt in range(DT):
                        nc.tensor.matmul(
                            ps,
                            lhsT=xT_sb[:, dt, tt * P:(tt + 1) * P],
                            rhs=wv_sb[:, dt, :],
                            start=(dt == 0), stop=(dt == DT - 1),
                        )
                    nc.vector.tensor_tensor(
                        v_sb[:, tt, :].rearrange("p (h c) -> p h c", c=HD + 1)[:, :, :HD],
                        ps.rearrange("p (h c) -> p h c", c=HD),
                        bv_bc.rearrange("p (h c) -> p h c", c=HD),
                        mybir.AluOpType.add,
                    )

            def finalize_qt(qt):
                xr = work.tile([P, GD], f32, tag="xr", name="xr")
                nc.sync.dma_start(xr, xres_d[qt * P:(qt + 1) * P, :])
                nc.vector.tensor_add(out_sb[:, qt, :], out_sb[:, qt, :], xr)
                nc.sync.dma_start(out_d[qt * P:(qt + 1) * P, :], out_sb[:, qt, :])

            def emit_head(hh, finalize=False):
                """Attention for core-local head hh (0..3)."""
                m, base = hh // 2, (hh % 2) * HD
                vcol = hh * (HD + 1)
                for qb in range(NQB):
                    q0 = qb * QB
                    exps = exps_pool.tile([P, KT, QB], bf16, tag="exps", name="exps")
                    for kt2 in range(KT // 2):
                        pss = psum.tile([P, 1024], f32, tag="ps_big", bufs=3, name="ps_s")
                        for half in range(2):
                            kt = 2 * kt2 + half
                            nc.tensor.matmul(
                                pss[:, half * QB:(half + 1) * QB],
                                lhsT=kT_sb[:, hh, kt * P:(kt + 1) * P],
                                rhs=qT_sb[:, m, q0:q0 + QB],
                                start=True, stop=True,
                            )
                        nc.scalar.activation(
                            out=exps[:, 2 * kt2:2 * kt2 + 2, :],
                            in_=pss, func=EXP, scale=0.125,
                        )
                    pso = psum.tile([HD + 1, QB], f32, tag="ps_small", name="ps_o")
                    for kt in range(KT):
                        nc.tensor.matmul(
                            pso,
                            lhsT=v_sb[:, kt, vcol:vcol + HD + 1],
                            rhs=exps[:, kt, :],
                            start=(kt == 0), stop=(kt == KT - 1),
                        )
                    oT = work.tile([HD + 1, QB], f32, tag="oT", name="oT")
                    nc.vector.tensor_copy(oT, pso)
                    if debug and hh == 0 and qb == 0:
                        nc.sync.dma_start(dbg_e[:, :, :], exps)
                        nc.sync.dma_start(dbg_o[:, :], oT)
                    for q4 in range(QB // P):
                        qt = qb * (QB // P) + q4
                        pst = psum.tile([P, HD + 1], f32, tag="ps_small", name="ps_t")
                        nc.tensor.transpose(
                            pst, oT[:, q4 * P:(q4 + 1) * P],
                            identity[:HD + 1, :HD + 1],
                        )
                        r = work.tile([P, 1], f32, tag="recip", name="recip")
                        nc.vector.reciprocal(r, pst[:, HD:HD + 1])
                        nc.vector.tensor_scalar_mul(
                            out_sb[:, qt, hh * HD:(hh + 1) * HD], pst[:, :HD], r,
                        )
                    if finalize:
                        # all other heads already wrote this q-block:
                        # residual-add + store right away
                        for q4 in range(QB // P):
                            finalize_qt(qb * (QB // P) + q4)

            # head 0 S-matmuls only need Q/K M-tile 0 -> exp starts early;
            # V and QK M-tile 1 projections fill PE slack under the
            # ACT-bound attention phases
            emit_qk(0)
            emit_v()
            emit_head(0)
            emit_qk(1)
            emit_head(1)
            emit_head(2)
            emit_head(3, finalize=True)

            if debug:
                nc.sync.dma_start(dbg_q[:, :, :], qT_sb)
                nc.sync.dma_start(dbg_k[:, :, :], kT_sb)
                nc.sync.dma_start(dbg_v[:, :, :], v_sb)

    nc.finalize()
    return nc


def _get_nc(debug=False):
    key = "nc_dbg" if debug else "nc"
    if key not in _CACHE:
        _CACHE[key] = _build_nc(debug=debug)
    return _CACHE[key]


def _round_fp32r(a):
    """Round fp32 to the fp32r grid (11-bit mantissa; low 12 bits dropped,
    round-half-up) so DMA'd data matches what the PE consumes as fp32r."""
    u = np.ascontiguousarray(a, dtype=np.float32).view(np.uint32)
    r = ((u.astype(np.uint64) + 0x800) & 0xFFFFF000).astype(np.uint32)
    return r.view(np.float32)


def kernel(x, Wq, bq, Wk, bk, Wv, bv):
    global LAST_RESULTS
    from concourse.bass_utils import run_bass_kernel_spmd

    x = np.asarray(x, dtype=np.float32)
    Wq, Wk, Wv = (np.asarray(a, dtype=np.float32) for a in (Wq, Wk, Wv))
    bq, bk, bv = (np.asarray(a, dtype=np.float32) for a in (bq, bk, bv))

    xTs = [x[b].T for b in range(B)]
    in_maps = []
    for c in range(NCORES):
        b, g = c // 4, c % 4
        cols = slice(GD * g, GD * (g + 1))
        xw = np.concatenate(
            [Wq[:, cols], Wk[:, cols], Wv[:, cols], xTs[b]],
            axis=1).astype(ml_dtypes.bfloat16)
        in_maps.append({
            "xw": xw,
            "bq": np.ascontiguousarray(bq[cols]),
            "bk": np.ascontiguousarray(bk[cols]),
            "bv": np.ascontiguousarray(bv[cols]),
            "xres": np.ascontiguousarray(x[b][:, cols]),
        })

    nc = _get_nc()
    res = run_bass_kernel_spmd(
        nc, in_maps, core_ids=list(range(NCORES)), trace=TRACE,
    )
    LAST_RESULTS = res

    full = np.empty((B, S, D), dtype=np.float32)
    for c in range(NCORES):
        b, g = c // 4, c % 4
        full[b, :, GD * g:GD * (g + 1)] = res.results[c]["out"]
    return full
